# revision 12
# baseline (speedup 1.0000x reference)
"""Trainium2 Bass kernel for nn_CrossAttention_LR_65249143160950.

Reference computation (per batch element):
    xf = x[b].reshape(C, N).T                      # [N, C] tokens
    xn = LayerNorm(xf) * gamma_norm
    q  = xn @ W_q                                  # [N, INNER]
    k, v = split(context[b] @ W_kv)                # [M, INNER] each
    per head: keys = [null_k, k_h, q_h], vals = [null_v, v_h, q_h]
    att = softmax(scale * q_h @ keys^T) @ vals
    out = LayerNorm(att_merged @ W_out) * gamma_out
    y[b] = (xf + out).T

Sharding: pure data parallelism — 16 batch elements / 8 cores = 2 per core.

Key algebraic simplifications used:
  * LayerNorm is invariant to a positive per-token scale, and the attention
    output feeds straight into LN(att @ W_out).  The softmax denominator is a
    per-token positive scalar, so the normalization (and the max-subtraction,
    safe here because |scale*sim| stays tiny) is skipped entirely: softmax
    becomes a bare exp.
  * gamma_norm is folded into W_q (LN's gamma only feeds the q projection).
  * All matmuls run in float32r (full PE rate, ~1e-4 relative error).

Layout: activations are feature-major ([feature-on-partitions, tokens-free]),
which makes every projection a natural PE matmul and makes softmax-exp/AV
transpose-free; LN statistics use PE column-sum matmuls with a 1/C ones
vector, and per-token stats are broadcast across partitions with K=1 rank-1
PE matmuls (which also fold gamma_out in as an outer product).
"""

import numpy as np

import concourse.bass as bass
import concourse.mybir as mybir
import concourse.tile as tile
from concourse import bass_utils
from concourse.masks import make_identity

F32 = mybir.dt.float32
F32R = mybir.dt.float32r
BF16 = mybir.dt.bfloat16
AF = mybir.ActivationFunctionType

N_CORES = 8
B_FULL, C, W, Hh = 16, 512, 32, 32
N = W * Hh                 # 1024 tokens
M, CTX = 77, 768
HEADS, DHEAD = 8, 64
INNER = HEADS * DHEAD      # 512
B = B_FULL // N_CORES      # 2 batch elements per core
SCALE = DHEAD ** -0.5
EPS = 1e-5
CK = C // 128              # 4 C-chunks
IK = INNER // 128          # 4 inner-chunks
XK = CTX // 128            # 6 ctx-chunks
NHALF = N // 512           # 2 token-halves
JB = 1 + N // 128          # 9 key blocks (block 0 = null+ctx+pad)
CTXP = 1 + M               # 78 used rows in block 0


def _split_sync_waits(nc):
    """This walrus build rejects >1 sync wait per instruction; move excess
    waits onto same-engine NoOps inserted immediately before the owner."""
    ctr = 0
    for f in nc.m.functions:
        for bb in f.blocks:
            insts = bb.instructions  # live list
            out = []
            changed = False
            for inst in insts:
                si = inst.sync_info
                waits = list(si.on_wait) if si is not None else []
                if len(waits) > 1:
                    changed = True
                    for w in waits[:-1]:
                        nop = mybir.InstNoOp(name=f"bass_waitsplit_{ctr}", ins=[], outs=[])
                        ctr += 1
                        nop.engine = inst.engine
                        nop.sync_info = mybir.SyncInfo(on_wait=[w], on_update=[])
                        out.append(nop)
                    inst.sync_info = mybir.SyncInfo(
                        on_wait=waits[-1:], on_update=list(si.on_update)
                    )
                out.append(inst)
            if changed:
                insts[:] = out
    return ctr


def build(n_reps=1):
    nc = bass.Bass("TRN2", target_bir_lowering=False, debug=False, num_devices=1)

    x_d = nc.dram_tensor("x", [B, C, W, Hh], F32, kind="ExternalInput").ap()
    ctx_d = nc.dram_tensor("context", [B, M, CTX], F32, kind="ExternalInput").ap()
    gn_d = nc.dram_tensor("gamma_norm", [C], F32, kind="ExternalInput").ap()
    nkv_d = nc.dram_tensor("null_kv", [2, DHEAD], F32, kind="ExternalInput").ap()
    wq_d = nc.dram_tensor("W_q", [C, INNER], F32, kind="ExternalInput").ap()
    wkv_d = nc.dram_tensor("W_kv", [CTX, 2 * INNER], F32, kind="ExternalInput").ap()
    wo_d = nc.dram_tensor("W_out", [INNER, C], F32, kind="ExternalInput").ap()
    go_d = nc.dram_tensor("gamma_out", [C], F32, kind="ExternalInput").ap()
    y_d = nc.dram_tensor("y", [B, C, W, Hh], F32, kind="ExternalOutput").ap()

    xv = x_d.rearrange("b (kb p) w h -> b kb p (w h)", p=128)
    yv = y_d.rearrange("b (kb p) w h -> b kb p (w h)", p=128)

    from contextlib import ExitStack

    with tile.TileContext(nc) as tc, ExitStack() as stack:
        singles = stack.enter_context(tc.tile_pool(name="singles", bufs=1))
        big = stack.enter_context(tc.tile_pool(name="big", bufs=1))
        dbl = stack.enter_context(tc.tile_pool(name="dbl", bufs=2))
        uno = stack.enter_context(tc.tile_pool(name="uno", bufs=1))
        epool = stack.enter_context(tc.tile_pool(name="epool", bufs=8))
        psum = stack.enter_context(tc.tile_pool(name="psum", bufs=8, space="PSUM"))

        # ---- one-time constants -------------------------------------------
        ident_f = singles.tile([128, 128], F32)
        make_identity(nc, ident_f)
        ident = singles.tile([128, 128], F32R)
        nc.vector.tensor_copy(out=ident, in_=ident_f)

        ones_inv = singles.tile([128, 1], BF16)      # 1/C for bf16 stat sums
        nc.vector.memset(ones_inv, 1.0 / C)
        ones_invr = singles.tile([128, 1], F32R)     # 1/C for f32r stat sums
        nc.vector.memset(ones_invr.bitcast(F32), 1.0 / C)
        ones_row = singles.tile([1, 128], F32R)      # broadcast lhsT
        nc.vector.memset(ones_row.bitcast(F32), 1.0)
        eps_t = singles.tile([1, 1], F32)
        nc.vector.memset(eps_t, EPS)
        ones64 = singles.tile([128, DHEAD], BF16)   # denominator-broadcast lhsT
        nc.vector.memset(ones64, 1.0)

        gam_n = singles.tile([128, CK], F32)
        nc.sync.dma_start(out=gam_n, in_=gn_d.rearrange("(kb p) -> p kb", p=128))
        go_row = singles.tile([1, C], F32R)          # gamma_out as a row
        nc.gpsimd.dma_start(out=go_row, in_=go_d.rearrange("(o c) -> o c", o=1))

        nk = singles.tile([DHEAD, 1], F32R)          # null key, feature-major
        nc.gpsimd.dma_start(out=nk, in_=nkv_d[0:1, :].rearrange("o d -> d o"))
        nv = singles.tile([1, DHEAD], F32R)          # null value row
        nc.gpsimd.dma_start(out=nv, in_=nkv_d[1:2, :])

        # W_q with gamma_norm folded in, [K-chunk partitions, inner free]
        wq = singles.tile([128, CK, INNER], F32R)
        nc.gpsimd.dma_start(out=wq, in_=wq_d.rearrange("(kb p) i -> p kb i", p=128))
        for kb in range(CK):
            nc.vector.tensor_scalar_mul(
                out=wq[:, kb, :], in0=wq[:, kb, :], scalar1=gam_n[:, kb : kb + 1]
            )
        wkv = singles.tile([128, XK, 2 * INNER], F32R)
        nc.gpsimd.dma_start(out=wkv, in_=wkv_d.rearrange("(kb p) i -> p kb i", p=128))
        wo = singles.tile([128, IK, C], F32R)
        nc.gpsimd.dma_start(out=wo, in_=wo_d.rearrange("(kb p) c -> p kb c", p=128))

        for rep in range(n_reps):
            for b in range(B):
                # ===== Stage A: load + pre-LN ==============================
                xf = dbl.tile([128, CK, N], F32, tag="xf")
                nc.sync.dma_start(out=xf, in_=xv[b].rearrange("kb p n -> p kb n"))

                xc = big.tile([128, CK, N], F32R, tag="xc")
                for nh in range(NHALF):
                    nsl = bass.ts(nh, 512)
                    mu_ps = psum.tile([1, 512], F32, tag="bank")
                    s2_ps = psum.tile([1, 512], F32, tag="bank")
                    for kb in range(CK):
                        xrc = dbl.tile([128, 512], BF16, tag="xrc")
                        nc.gpsimd.tensor_copy(out=xrc, in_=xf[:, kb, nsl])
                        nc.tensor.matmul(
                            mu_ps, ones_inv, xrc,
                            start=(kb == 0), stop=(kb == CK - 1),
                        )
                        sq = dbl.tile([128, 512], BF16, tag="sq")
                        nc.gpsimd.tensor_mul(out=sq, in0=xrc, in1=xrc)
                        nc.tensor.matmul(
                            s2_ps, ones_inv, sq,
                            start=(kb == 0), stop=(kb == CK - 1),
                        )
                    mu_sb = dbl.tile([1, 512], F32R, tag="murow")
                    nc.vector.tensor_copy(out=mu_sb, in_=mu_ps)
                    musq = dbl.tile([1, 512], F32, tag="musq")
                    nc.vector.tensor_mul(out=musq, in0=mu_sb, in1=mu_sb)
                    var = dbl.tile([1, 512], F32, tag="var")
                    nc.vector.tensor_sub(out=var, in0=s2_ps, in1=musq)
                    lnv = dbl.tile([1, 512], F32, tag="lnv")
                    nc.scalar.activation(out=lnv, in_=var, func=AF.Ln, bias=eps_t)
                    rstd = dbl.tile([1, 512], F32R, tag="rstd")
                    nc.scalar.activation(out=rstd, in_=lnv, func=AF.Exp, scale=-0.5)

                    mub = psum.tile([128, 512], F32, tag="bank")
                    nc.tensor.matmul(mub, ones_row, mu_sb, start=True, stop=True)
                    rb = psum.tile([128, 512], F32, tag="bank")
                    nc.tensor.matmul(rb, ones_row, rstd, start=True, stop=True)
                    for kb in range(CK):
                        t1 = dbl.tile([128, 512], F32, tag="t1")
                        nc.vector.tensor_sub(out=t1, in0=xf[:, kb, nsl], in1=mub)
                        nc.vector.tensor_mul(out=xc[:, kb, nsl], in0=t1, in1=rb)

                # ===== Stage B: projections ================================
                qT = big.tile([128, IK, N], F32R, tag="qT")
                for it in range(IK):
                    for nh in range(NHALF):
                        q_ps = psum.tile([128, 512], F32, tag="bank")
                        for kb in range(CK):
                            nc.tensor.matmul(
                                q_ps, wq[:, kb, bass.ts(it, 128)],
                                xc[:, kb, bass.ts(nh, 512)],
                                start=(kb == 0), stop=(kb == CK - 1),
                            )
                        nc.vector.tensor_copy(out=qT[:, it, bass.ts(nh, 512)], in_=q_ps)

                ctx_sb = uno.tile([M, CTX], F32R, tag="ctx")
                nc.gpsimd.dma_start(out=ctx_sb, in_=ctx_d[b])
                ctxT = uno.tile([128, XK, M], F32R, tag="ctxT")
                for xb in range(XK):
                    tp = psum.tile([128, CTXP], F32R, tag="bank")
                    nc.tensor.transpose(
                        tp, ctx_sb[:, bass.ts(xb, 128)], ident[0:M, 0:CTXP]
                    )
                    nc.vector.tensor_copy(out=ctxT[:, xb, :], in_=tp[:, 0:M])

                kctx = uno.tile([M, INNER], F32R, tag="kctx")
                vctx = uno.tile([M, INNER], BF16, tag="vctx")
                for half, dst in ((0, kctx), (1, vctx)):
                    kv_ps = psum.tile([M, 512], F32, tag="bank")
                    for xb in range(XK):
                        nc.tensor.matmul(
                            kv_ps, ctxT[:, xb, :],
                            wkv[:, xb, bass.ts(half, INNER)],
                            start=(xb == 0), stop=(xb == XK - 1),
                        )
                    nc.vector.tensor_copy(out=dst, in_=kv_ps)

                # keys block 0 per head-pair: [null | ctx^T | zero-pad]
                k0 = uno.tile([128, HEADS // 2, 128], F32R, tag="k0")
                v0 = uno.tile([128, HEADS, DHEAD], BF16, tag="v0")
                for h in range(HEADS):
                    p, hh = divmod(h, 2)
                    rsl = slice(hh * 64, hh * 64 + 64)
                    nc.gpsimd.memset(k0[rsl, p, CTXP:128].bitcast(F32), 0.0)
                    nc.gpsimd.dma_start(
                        out=k0[rsl, p, 0:1], in_=nk
                    )
                    tk = psum.tile([DHEAD, CTXP], F32R, tag="bank")
                    nc.tensor.transpose(
                        tk, kctx[:, bass.ts(h, DHEAD)], ident[0:M, 0:CTXP]
                    )
                    tks = uno.tile([DHEAD, M], F32R, tag="tks")
                    nc.vector.tensor_copy(out=tks, in_=tk[:, 0:M])
                    nc.gpsimd.dma_start(out=k0[rsl, p, 1:CTXP], in_=tks)
                    # values block 0, token-major: zero pad rows first
                    # (aligned memset), then overwrite with null+ctx values
                    nc.gpsimd.memset(v0[64:128, h, :], 0.0)
                    nc.vector.tensor_copy(out=v0[0:1, h, :], in_=nv)
                    nc.gpsimd.dma_start(out=v0[1:CTXP, h, :], in_=vctx[:, bass.ts(h, DHEAD)])

                # ===== Stage C: attention ==================================
                oT = big.tile([128, IK, N], F32R, tag="oT")
                for p in range(HEADS // 2):
                    vq = dbl.tile([128, 2 * (N // 128), DHEAD], BF16, tag="vq")
                    for hh in range(2):
                        rsl = slice(hh * 64, hh * 64 + 64)
                        for ib in range(N // 128):
                            tq = psum.tile([128, DHEAD], F32R, tag="bank")
                            nc.tensor.transpose(
                                tq, qT[rsl, p, bass.ts(ib, 128)], ident[rsl, rsl]
                            )
                            nc.vector.tensor_copy(out=vq[:, hh * 8 + ib, :], in_=tq)
                    for ih in range(NHALF):
                        isl = bass.ts(ih, 512)
                        av_ps = psum.tile([128, 512], F32, tag="bank")
                        db_ps = psum.tile([128, 512], F32, tag="bank")
                        for jb in range(JB):
                            for hh in range(2):
                                h = 2 * p + hh
                                rsl = slice(hh * 64, hh * 64 + 64)
                                sim_ps = psum.tile([128, 512], F32, tag="bank")
                                lhs = (
                                    k0[rsl, p, :]
                                    if jb == 0
                                    else qT[rsl, p, bass.ts(jb - 1, 128)]
                                )
                                nc.tensor.matmul(
                                    sim_ps, lhs, qT[rsl, p, isl],
                                    start=True, stop=True,
                                )
                                eb = epool.tile([128, 512], BF16, tag="E")
                                if jb == 0:
                                    nc.gpsimd.memset(eb[64:128, :], 0.0)
                                    nc.scalar.activation(
                                        out=eb[0:CTXP, :], in_=sim_ps[0:CTXP, :],
                                        func=AF.Exp, scale=SCALE,
                                    )
                                else:
                                    nc.scalar.activation(
                                        out=eb, in_=sim_ps, func=AF.Exp, scale=SCALE
                                    )
                                nc.tensor.matmul(
                                    av_ps[rsl, :],
                                    v0[:, h, :] if jb == 0 else vq[:, hh * 8 + jb - 1, :],
                                    eb,
                                    start=(jb == 0), stop=(jb == JB - 1),
                                )
                                # per-head softmax denominator, broadcast to
                                # all 64 head dims by an all-ones stationary
                                nc.tensor.matmul(
                                    db_ps[rsl, :], ones64, eb,
                                    start=(jb == 0), stop=(jb == JB - 1),
                                )
                        rd = dbl.tile([128, 512], F32R, tag="rd")
                        with nc.allow_low_precision(reason="f32r rounding of 1/denom is intentional"):
                            nc.vector.reciprocal(out=rd, in_=db_ps)
                        nc.vector.tensor_mul(out=oT[:, p, isl], in0=av_ps, in1=rd)

                # ===== Stage D: out projection + post-LN + residual ========
                osb = big.tile([128, CK, N], F32R, tag="xc")  # reuses xc slot (disjoint lifetime)
                for ct in range(CK):
                    for nh in range(NHALF):
                        z_ps = psum.tile([128, 512], F32, tag="bank")
                        for kb in range(IK):
                            nc.tensor.matmul(
                                z_ps, wo[:, kb, bass.ts(ct, 128)],
                                oT[:, kb, bass.ts(nh, 512)],
                                start=(kb == 0), stop=(kb == IK - 1),
                            )
                        nc.vector.tensor_copy(out=osb[:, ct, bass.ts(nh, 512)], in_=z_ps)

                for nh in range(NHALF):
                    nsl = bass.ts(nh, 512)
                    mu_ps = psum.tile([1, 512], F32, tag="bank")
                    for kb in range(CK):
                        nc.tensor.matmul(
                            mu_ps, ones_invr, osb[:, kb, nsl],
                            start=(kb == 0), stop=(kb == CK - 1),
                        )
                    s2_ps = psum.tile([1, 512], F32, tag="bank")
                    for kb in range(CK):
                        sq = dbl.tile([128, 512], F32R, tag="sq2")
                        nc.gpsimd.tensor_mul(
                            out=sq, in0=osb[:, kb, nsl], in1=osb[:, kb, nsl]
                        )
                        nc.tensor.matmul(
                            s2_ps, ones_invr, sq,
                            start=(kb == 0), stop=(kb == CK - 1),
                        )
                    mu_sb = dbl.tile([1, 512], F32R, tag="murow")
                    nc.vector.tensor_copy(out=mu_sb, in_=mu_ps)
                    musq = dbl.tile([1, 512], F32, tag="musq")
                    nc.vector.tensor_mul(out=musq, in0=mu_sb, in1=mu_sb)
                    var = dbl.tile([1, 512], F32, tag="var")
                    nc.vector.tensor_sub(out=var, in0=s2_ps, in1=musq)
                    lnv = dbl.tile([1, 512], F32, tag="lnv")
                    nc.scalar.activation(out=lnv, in_=var, func=AF.Ln, bias=eps_t)
                    rstd = dbl.tile([1, 512], F32R, tag="rstd")
                    nc.scalar.activation(out=rstd, in_=lnv, func=AF.Exp, scale=-0.5)

                    mub = psum.tile([128, 512], F32, tag="bank")
                    nc.tensor.matmul(mub, ones_row, mu_sb, start=True, stop=True)
                    for ct in range(CK):
                        # gamma_out[c] * rstd[t] as a rank-1 PE outer product
                        grb = psum.tile([128, 512], F32, tag="bank")
                        nc.tensor.matmul(
                            grb, go_row[:, bass.ts(ct, 128)], rstd,
                            start=True, stop=True,
                        )
                        t1 = dbl.tile([128, 512], F32, tag="t1")
                        nc.vector.tensor_sub(out=t1, in0=osb[:, ct, nsl], in1=mub)
                        t2 = dbl.tile([128, 512], F32, tag="t2")
                        nc.vector.tensor_mul(out=t2, in0=t1, in1=grb)
                        yt = dbl.tile([128, 512], F32, tag="yout")
                        nc.vector.tensor_add(out=yt, in0=t2, in1=xf[:, ct, nsl])
                        nc.sync.dma_start(out=yv[b, ct, :, nsl], in_=yt)

    _split_sync_waits(nc)
    return nc


_NC_CACHE = {}


def _get_nc(n_reps=1):
    if n_reps not in _NC_CACHE:
        _NC_CACHE[n_reps] = build(n_reps)
    return _NC_CACHE[n_reps]


def kernel(**inputs):
    x = np.ascontiguousarray(np.asarray(inputs["x"], dtype=np.float32))
    context = np.ascontiguousarray(np.asarray(inputs["context"], dtype=np.float32))
    shared = {
        k: np.ascontiguousarray(np.asarray(inputs[k], dtype=np.float32))
        for k in ("gamma_norm", "null_kv", "W_q", "W_kv", "W_out", "gamma_out")
    }
    nc = _get_nc()
    in_maps = []
    for c in range(N_CORES):
        sl = slice(c * B, (c + 1) * B)
        in_maps.append({"x": x[sl], "context": context[sl], **shared})
    res = bass_utils.run_bass_kernel_spmd(nc, in_maps, core_ids=list(range(N_CORES)))
    out = np.concatenate([res.results[c]["y"] for c in range(N_CORES)], axis=0)
    return out.astype(np.float32)
